# revision 24
# baseline (speedup 1.0000x reference)
"""Trainium2 Bass kernel for nn_Attention_3556232921308.

GQA attention layer: RMSNorm -> {Q+gate, K, V} proj -> softmax attention
(no mask, no rope) -> sigmoid output gate -> O proj.
B=2, S=2048, HID=2048, NH=16, NKV=4, HD=128.

Sharding (8 cores): DP over batch (2 groups of 4 cores) x TP over KV heads
(4 ranks per group; each rank owns 1 KV head = 4 Q/gate heads). The output
projection contracts over all heads, so gated attention outputs (bf16) are
exchanged with per-head AllGathers; each rank then computes the O-projection
for all tokens but only its quarter of the HID output columns (the Wo slice
is host-provided per rank, keeping the SPMD graph rank-independent).

Host-side prep: transposes (activations/weights enter the PE contracted
over the partition dim) and folding the RMSNorm (1+w) scale into the
projection weights. Matmuls run as float32r (1 cycle/row on TRN2 for moving
dim >= 256, ~1e-4 rel err); the O-projection runs bf16.

Compute layout notes:
 - hidden states live as hsT [HID, S]; mean-square is a ones-matvec on PE,
   and 1/rms is applied to the *outputs* of the raw projections (scaling by
   rstd commutes past the HID contraction), broadcast along partitions.
 - q/k are produced directly in [HD, S] (per head) layout, so scores^T
   [Sk, Sq] needs no transposes; softmax denominators are ones-matvecs.
 - v is produced as vT [HD, S] at full rate then PE-transposed per 128-tile.
 - exp(scores) runs on ACT straight out of PSUM with the 1/sqrt(HD) scale
   folded in; no max-subtraction (|scores| < 8 for unit-RMS inputs and
   0.02-scaled weights; fp32 exp is safe). The sigmoid gate is computed as
   1/(1+exp(-g)) so ACT never swaps activation tables in the hot loop.
"""
import math
from contextlib import ExitStack

import numpy as np

B, S_FULL, HID = 2, 2048, 2048
NH, NKV, HD = 16, 4, 128
G = NH // NKV  # 4 q heads per kv head = heads per rank
EPS = 1e-6
N_CORES = 8
P = 128
KH = HID // P  # 16 contraction tiles
HQ = HID // 4  # per-rank output column quarter (512)


def build(S=S_FULL):
    import concourse.bass as bass  # noqa: F401
    import concourse.tile as tile
    from concourse import bacc, mybir

    F32R = mybir.dt.float32r
    F32 = mybir.dt.float32
    BF16 = mybir.dt.bfloat16
    AF = mybir.ActivationFunctionType

    SQCH = S // 4  # attention sq chunk
    NW = min(512, S)  # projection free-dim chunk
    NCH = S // NW
    NSK = S // P  # score key tiles
    HPR = G * HD  # feats per rank for q/gate (512)
    SCALE = 1.0 / math.sqrt(HD)
    RG = [[0, 1, 2, 3], [4, 5, 6, 7]]

    nc = bacc.Bacc("TRN2", target_bir_lowering=False, debug=False, num_devices=N_CORES)

    hst = nc.declare_dram_parameter("hst", [HID, S], F32R, isOutput=False)
    wqt = nc.declare_dram_parameter("wqt", [HID, 2 * HPR], F32R, isOutput=False)
    wkt = nc.declare_dram_parameter("wkt", [HID, HD], F32R, isOutput=False)
    wvt = nc.declare_dram_parameter("wvt", [HID, HD], F32R, isOutput=False)
    wot = nc.declare_dram_parameter("wot", [NH * HD, HQ], F32R, isOutput=False)
    onesp = nc.declare_dram_parameter("onesp", [P, 1], F32R, isOutput=False)
    identp = nc.declare_dram_parameter("identp", [P, P], F32R, isOutput=False)
    out = nc.declare_dram_parameter("out", [HQ, S], F32, isOutput=True)

    with tile.TileContext(nc) as tc, ExitStack() as ctx:
        dram = ctx.enter_context(tc.tile_pool(name="dram", bufs=1, space="DRAM"))
        qt_dram = dram.tile([G * P, S], F32R)
        gate_dram = dram.tile([G * P, S], F32R)
        ag_in = [
            dram.tile([P, S], BF16, name=f"ag_in{h}", uniquify=False)
            for h in range(G)
        ]
        ag_out = [
            dram.tile([4 * P, S], BF16, name=f"ag_out{h}", uniquify=False)
            for h in range(G)
        ]

        # tiny warmup collective: absorbs NRT collective-channel init +
        # cross-core launch skew concurrently with the compute phases, so
        # the first real AllGather runs at steady-state cost
        # same byte-size as the real per-head AllGathers so the mesh channel
        # setup for that message size is paid here, overlapped with compute
        warm_in = dram.tile([P, S // 2], F32R)
        warm_out = dram.tile([4 * P, S // 2], F32R)
        nc.gpsimd.dma_start(out=warm_in[:], in_=hst[0:P, 0:S // 2])
        nc.gpsimd.collective_compute(
            "AllGather",
            mybir.AluOpType.bypass,
            replica_groups=RG,
            ins=[warm_in[:].opt()],
            outs=[warm_out[:].opt()],
        )
        rs_dram = dram.tile([1, S // 4], F32)

        consts = ctx.enter_context(tc.tile_pool(name="consts", bufs=1))
        ones_sb = consts.tile([P, 1], F32R)
        nc.sync.dma_start(out=ones_sb[:], in_=onesp[:])
        ident_sb = consts.tile([P, P], F32R)
        nc.sync.dma_start(out=ident_sb[:], in_=identp[:])
        rstd_bc = consts.tile([P, S], F32)
        eps_t = consts.tile([1, 1], F32)
        nc.vector.memset(eps_t[:], EPS)

        with ExitStack() as ph123:
            kv_pool = ph123.enter_context(tc.tile_pool(name="kv", bufs=1))
            kt_sb = kv_pool.tile([P, S], F32R)
            vnat = [kv_pool.tile([P, P], F32R, name=f"vnat{i}", uniquify=False)
                    for i in range(NSK)]

            # ---- phases 1+2: norm stats + projections (hsT resident) ----
            with ExitStack() as ph:
                ht_pool = ph.enter_context(tc.tile_pool(name="ht", bufs=1))
                ht = [ht_pool.tile([P, S], F32R, name=f"ht{k}", uniquify=False)
                      for k in range(KH)]
                vt_sb = ht_pool.tile([P, S], F32R)
                for k in range(KH):
                    dma_eng = nc.sync if k % 2 == 0 else nc.scalar
                    dma_eng.dma_start(out=ht[k][:], in_=hst[k * P:(k + 1) * P, :])

                # norm stats + projections. The mean-square matvecs are
                # software-pipelined one k behind their squares (half on
                # ACT, half on DVE), with the m=0 q-projection psum chains
                # interleaved so the PE never idles waiting on squares.
                with tc.tile_pool(name="sqp", bufs=4) as sqp, tc.tile_pool(
                    name="wq", bufs=2
                ) as wqp, tc.tile_pool(name="ev", bufs=3) as evp:

                    def load_wq(m):
                        wq_m = wqp.tile([P, KH, P], F32R, name="wq_m", tag="wq_m")
                        for k in range(KH):
                            nc.sync.dma_start(
                                out=wq_m[:, k, :],
                                in_=wqt[k * P:(k + 1) * P, m * P:(m + 1) * P],
                            )
                        return wq_m

                    def qg_chain_mm(ps, wq_m, k, n):
                        nc.tensor.matmul(
                            ps[:],
                            wq_m[:, k, :],
                            ht[k][:, n * NW:(n + 1) * NW],
                            start=(k == 0),
                            stop=(k == KH - 1),
                        )

                    def evac(ps, m, n):
                        ev = evp.tile([P, NW], F32R)
                        nc.vector.tensor_mul(
                            ev[:], ps[:], rstd_bc[:, n * NW:(n + 1) * NW]
                        )
                        dst = qt_dram if m < G else gate_dram
                        mm = m if m < G else m - G
                        nc.sync.dma_start(
                            out=dst[mm * P:(mm + 1) * P, n * NW:(n + 1) * NW],
                            in_=ev[:],
                        )

                    qgps0_cm = tc.tile_pool(name="qgps0", bufs=1, space="PSUM")
                    qgps0 = qgps0_cm.__enter__()
                    with tc.tile_pool(name="msp", bufs=1, space="PSUM") as msp:
                        ms_ps = [msp.tile([1, NW], F32, name=f"ms{n}",
                                          uniquify=False) for n in range(NCH)]
                        wq_0 = load_wq(0)
                        ps_m0 = [qgps0.tile([P, NW], F32, name=f"psq{n}")
                                 for n in range(NCH)]
                        sq_prev = None
                        for k in range(KH):
                            sq_k = []
                            for n in range(NCH):
                                sqk = sqp.tile([P, NW], F32R)
                                src = ht[k][:, n * NW:(n + 1) * NW]
                                if n % 2 == 0:
                                    nc.scalar.activation(sqk[:], src, AF.Square)
                                else:
                                    nc.vector.tensor_mul(sqk[:], src, src)
                                sq_k.append(sqk)
                            for n in range(NCH):
                                qg_chain_mm(ps_m0[n], wq_0, k, n)
                            if sq_prev is not None:
                                for n in range(NCH):
                                    nc.tensor.matmul(
                                        ms_ps[n][:],
                                        ones_sb[:],
                                        sq_prev[n][:],
                                        start=(k == 1),
                                        stop=(k == KH - 1 + 1),
                                    )
                            sq_prev = sq_k
                        for n in range(NCH):
                            nc.tensor.matmul(
                                ms_ps[n][:], ones_sb[:], sq_prev[n][:],
                                start=False, stop=True,
                            )
                        srow = sqp.tile([1, S], F32, bufs=1)
                        for n in range(NCH):
                            nc.scalar.activation(
                                srow[:, n * NW:(n + 1) * NW],
                                ms_ps[n][:],
                                AF.Sqrt,
                                bias=eps_t[:],
                                scale=1.0 / HID,
                            )
                        nc.vector.reciprocal(srow[:], srow[:])
                        nc.gpsimd.partition_broadcast(rstd_bc[:], srow[:])
                        for n in range(NCH):
                            evac(ps_m0[n], 0, n)
                    qgps0_cm.__exit__(None, None, None)

                    with tc.tile_pool(name="qgps", bufs=2, space="PSUM") as qgps:
                        # remaining q/gate head tiles
                        for m in range(1, 2 * G):
                            wq_m = load_wq(m)
                            for n in range(NCH):
                                ps = qgps.tile([P, NW], F32, name="psq_r")
                                for k in range(KH):
                                    qg_chain_mm(ps, wq_m, k, n)
                                evac(ps, m, n)

                    # k and v (vT), rstd-scaled at evacuation; the weight
                    # tiles share the wq_m rotation slots
                    wk_sb = wqp.tile([P, KH, P], F32R, name="wk_sb", tag="wq_m")
                    wv_sb = wqp.tile([P, KH, P], F32R, name="wv_sb", tag="wq_m")
                    for k in range(KH):
                        nc.sync.dma_start(
                            out=wk_sb[:, k, :], in_=wkt[k * P:(k + 1) * P, :]
                        )
                        nc.sync.dma_start(
                            out=wv_sb[:, k, :], in_=wvt[k * P:(k + 1) * P, :]
                        )
                    with tc.tile_pool(name="kvps", bufs=2, space="PSUM") as kvps:
                     for dst_sb, w_sb in ((kt_sb, wk_sb), (vt_sb, wv_sb)):
                        for n in range(NCH):
                            ps = kvps.tile([P, NW], F32, name="ps_kv")
                            for k in range(KH):
                                nc.tensor.matmul(
                                    ps[:],
                                    w_sb[:, k, :],
                                    ht[k][:, n * NW:(n + 1) * NW],
                                    start=(k == 0),
                                    stop=(k == KH - 1),
                                )
                            nc.vector.tensor_mul(
                                dst_sb[:, n * NW:(n + 1) * NW],
                                ps[:],
                                rstd_bc[:, n * NW:(n + 1) * NW],
                            )

                # v natural layout via PE transpose of vT tiles
                with tc.tile_pool(name="tpps", bufs=2, space="PSUM") as tpps:
                    for sk in range(NSK):
                        pst = tpps.tile([P, P], F32R)
                        nc.tensor.transpose(
                            pst[:], vt_sb[:, sk * P:(sk + 1) * P], ident_sb[:]
                        )
                        nc.vector.tensor_copy(vnat[sk][:], pst[:])

            # ---- phases 3+4 pools (allocated in the freed hsT zone) ----
            with ExitStack() as ph34:
                wo_bfp = ph34.enter_context(tc.tile_pool(name="wo_bf", bufs=1))
                wo_bf = [wo_bfp.tile([P, HQ], BF16, name=f"wo{kf}", uniquify=False)
                         for kf in range(KH)]
                of_pool = ph34.enter_context(tc.tile_pool(name="of", bufs=1))
                of = [of_pool.tile([P, S], BF16, name=f"of{i}", uniquify=False)
                      for i in range(KH)]
                with tc.tile_pool(name="wof", bufs=2) as wofp:
                    for kf in range(KH):
                        wof = wofp.tile([P, HQ], F32R)
                        nc.sync.dma_start(
                            out=wof[:], in_=wot[kf * P:(kf + 1) * P, :]
                        )
                        nc.vector.tensor_copy(wo_bf[kf][:], wof[:])

                # ---- phase 3: attention ----
                with tc.tile_pool(name="qload", bufs=3) as qlp, tc.tile_pool(
                    name="pt", bufs=3
                ) as ptp, tc.tile_pool(name="og", bufs=2) as ogp, tc.tile_pool(
                    name="sps", bufs=2, space="PSUM"
                ) as sps, tc.tile_pool(
                    name="ops", bufs=2, space="PSUM"
                ) as ops, tc.tile_pool(name="sums", bufs=2, space="PSUM") as sums:
                    for h in range(G):
                        for sqc in range(4):
                            ssl = slice(sqc * SQCH, (sqc + 1) * SQCH)
                            qtile = qlp.tile([P, SQCH], F32R)
                            nc.sync.dma_start(
                                out=qtile[:], in_=qt_dram[h * P:(h + 1) * P, ssl]
                            )
                            gtile = qlp.tile([P, SQCH], F32R)
                            nc.sync.dma_start(
                                out=gtile[:], in_=gate_dram[h * P:(h + 1) * P, ssl]
                            )
                            ps_o = ops.tile([P, SQCH], F32)
                            ps_sum = sums.tile([1, SQCH], F32)

                            # pairs of sk tiles share one 2-bank psum + one
                            # exp; p@v of pair skp-1 is emitted after the
                            # scores of pair skp so the PE never sits behind
                            # the ACT exp on the critical path
                            def emit_pv(skp, pt):
                                for j in range(2):
                                    sk = 2 * skp + j
                                    nc.tensor.matmul(
                                        ps_sum[:],
                                        ones_sb[:],
                                        pt[:, j, :],
                                        start=(sk == 0),
                                        stop=(sk == NSK - 1),
                                    )
                                    nc.tensor.matmul(
                                        ps_o[:],
                                        vnat[sk][:],
                                        pt[:, j, :],
                                        start=(sk == 0),
                                        stop=(sk == NSK - 1),
                                    )

                            pend = None
                            for skp in range(NSK // 2):
                                ps_s = sps.tile([P, 2, SQCH], F32)
                                for j in range(2):
                                    sk = 2 * skp + j
                                    nc.tensor.matmul(
                                        ps_s[:, j, :],
                                        kt_sb[:, sk * P:(sk + 1) * P],
                                        qtile[:],
                                        start=True,
                                        stop=True,
                                    )
                                pt = ptp.tile([P, 2, SQCH], F32R)
                                nc.scalar.activation(
                                    pt[:], ps_s[:], AF.Exp, scale=SCALE
                                )
                                if pend is not None:
                                    emit_pv(*pend)
                                pend = (skp, pt)
                            emit_pv(*pend)

                            # normalize + sigmoid gate (exp-form, no ACT
                            # table swap: sigmoid(g) = 1/(1+exp(-g)))
                            rs = ogp.tile([1, SQCH], F32)
                            nc.vector.reciprocal(rs[:], ps_sum[:])
                            # broadcast 1/sum along partitions via a DRAM
                            # round-trip on the sync DGE (gpsimd must stay
                            # free to trigger collectives)
                            nc.sync.dma_start(out=rs_dram[:, :SQCH], in_=rs[:])
                            rb = ogp.tile([P, SQCH], F32)
                            nc.sync.dma_start(
                                out=rb[:],
                                in_=rs_dram[:, :SQCH].to_broadcast((P, SQCH)),
                            )
                            eng = ogp.tile([P, SQCH], F32)
                            nc.scalar.activation(eng[:], gtile[:], AF.Exp, scale=-1.0)
                            e1 = ogp.tile([P, SQCH], F32)
                            nc.vector.tensor_scalar_add(e1[:], eng[:], 1.0)
                            sig = ogp.tile([P, SQCH], F32)
                            nc.vector.reciprocal(sig[:], e1[:])
                            # multiply by the gate FIRST: releases the ps_o
                            # psum slot without waiting on the rb DMA trip
                            t1 = ogp.tile([P, SQCH], F32)
                            nc.vector.tensor_mul(t1[:], ps_o[:], sig[:])
                            og = ogp.tile([P, SQCH], BF16)
                            nc.vector.tensor_mul(og[:], t1[:], rb[:])
                            nc.sync.dma_start(out=ag_in[h][:, ssl], in_=og[:])
                        nc.gpsimd.collective_compute(
                            "AllGather",
                            mybir.AluOpType.bypass,
                            replica_groups=RG,
                            ins=[ag_in[h][:].opt()],
                            outs=[ag_out[h][:].opt()],
                        )
                        # off the sync queue: these wait on the AllGather and
                        # must not head-of-line-block the q/gate reloads
                        for r in range(4):
                            nc.scalar.dma_start(
                                out=of[h * 4 + r][:],
                                in_=ag_out[h][r * P:(r + 1) * P, :],
                            )

                # ---- phase 4: O projection (bf16), my HID column quarter ----
                # kf-outer accumulation so heads 0..2 contract while the
                # last AllGather is still in flight
                with tc.tile_pool(name="outps", bufs=2, space="PSUM") as outps, \
                        tc.tile_pool(name="oev", bufs=3) as oevp:
                    NM = HQ // P
                    for n in range(NCH):
                        pss = [outps.tile([P, NW], F32, name=f"ops{m}")
                               for m in range(NM)]
                        for kf in range(KH):
                            for m in range(NM):
                                nc.tensor.matmul(
                                    pss[m][:],
                                    wo_bf[kf][:, m * P:(m + 1) * P],
                                    of[kf][:, n * NW:(n + 1) * NW],
                                    start=(kf == 0),
                                    stop=(kf == KH - 1),
                                )
                        for m in range(NM):
                            oev = oevp.tile([P, NW], F32)
                            nc.vector.tensor_copy(oev[:], pss[m][:])
                            nc.sync.dma_start(
                                out=out[m * P:(m + 1) * P, n * NW:(n + 1) * NW],
                                in_=oev[:],
                            )

    nc.compile()
    return nc


def make_in_maps(hidden_states, Wq, Wk, Wv, Wo, norm_w, S=S_FULL):
    """Host-side sharding/layout prep. Core c -> (batch c//4, rank c%4)."""
    w1p = (1.0 + norm_w).astype(np.float32)
    WqT = np.ascontiguousarray((Wq * w1p[None, :]).T)  # [HID, 2*NH*HD]
    WkT = np.ascontiguousarray((Wk * w1p[None, :]).T)  # [HID, NKV*HD]
    WvT = np.ascontiguousarray((Wv * w1p[None, :]).T)
    WoT = np.ascontiguousarray(Wo.T)  # [NH*HD, HID]
    # permute feat blocks to match AG stacking: pos h*4+r holds head 4r+h
    perm = [4 * (p % 4) + p // 4 for p in range(NH)]
    WoTp = np.ascontiguousarray(
        WoT.reshape(NH, HD, HID)[perm].reshape(NH * HD, HID)
    )
    ones = np.ones((P, 1), np.float32)
    ident = np.eye(P, dtype=np.float32)

    in_maps = []
    for c in range(N_CORES):
        b, r = c // 4, c % 4
        qcols = np.r_[r * 512:(r + 1) * 512, NH * HD + r * 512:NH * HD + (r + 1) * 512]
        in_maps.append(
            {
                "hst": np.ascontiguousarray(hidden_states[b, :S].T),
                "wqt": np.ascontiguousarray(WqT[:, qcols]),
                "wkt": np.ascontiguousarray(WkT[:, r * HD:(r + 1) * HD]),
                "wvt": np.ascontiguousarray(WvT[:, r * HD:(r + 1) * HD]),
                "wot": np.ascontiguousarray(WoTp[:, r * HQ:(r + 1) * HQ]),
                "onesp": ones,
                "identp": ident,
            }
        )
    return in_maps


def gather_out(results, S=S_FULL):
    out = np.empty((B, S, HID), np.float32)
    for c in range(N_CORES):
        b, r = c // 4, c % 4
        out[b, :, r * HQ:(r + 1) * HQ] = results[c]["out"].T
    return out


_NC_CACHE = {}


def kernel(**inputs) -> np.ndarray:
    from concourse.bass_utils import run_bass_kernel_spmd

    hidden_states = np.asarray(inputs["hidden_states"], dtype=np.float32)
    Wq = np.asarray(inputs["Wq"], dtype=np.float32)
    Wk = np.asarray(inputs["Wk"], dtype=np.float32)
    Wv = np.asarray(inputs["Wv"], dtype=np.float32)
    Wo = np.asarray(inputs["Wo"], dtype=np.float32)
    norm_w = np.asarray(inputs["norm_w"], dtype=np.float32)

    if "nc" not in _NC_CACHE:
        _NC_CACHE["nc"] = build()
    nc = _NC_CACHE["nc"]

    in_maps = make_in_maps(hidden_states, Wq, Wk, Wv, Wo, norm_w)
    res = run_bass_kernel_spmd(nc, in_maps, list(range(N_CORES)))
    return gather_out(res.results)


# revision 25
# speedup vs baseline: 1.0643x; 1.0643x over previous
"""Trainium2 Bass kernel for nn_Attention_3556232921308.

GQA attention layer: RMSNorm -> {Q+gate, K, V} proj -> softmax attention
(no mask, no rope) -> sigmoid output gate -> O proj.
B=2, S=2048, HID=2048, NH=16, NKV=4, HD=128.

Sharding (8 cores): DP over batch (2 groups of 4 cores) x TP over KV heads
(4 ranks per group; each rank owns 1 KV head = 4 Q/gate heads). The output
projection contracts over all heads, so gated attention outputs (bf16) are
exchanged with per-head AllGathers; each rank then computes the O-projection
for all tokens but only its quarter of the HID output columns (the Wo slice
is host-provided per rank, keeping the SPMD graph rank-independent).

Host-side prep: transposes (activations/weights enter the PE contracted
over the partition dim) and folding the RMSNorm (1+w) scale into the
projection weights. Matmuls run as float32r (1 cycle/row on TRN2 for moving
dim >= 256, ~1e-4 rel err); the O-projection runs bf16.

Compute layout notes:
 - hidden states live as hsT [HID, S]; mean-square is a ones-matvec on PE,
   and 1/rms is applied to the *outputs* of the raw projections (scaling by
   rstd commutes past the HID contraction), broadcast along partitions.
 - q/k are produced directly in [HD, S] (per head) layout, so scores^T
   [Sk, Sq] needs no transposes; softmax denominators are ones-matvecs.
 - v is produced as vT [HD, S] at full rate then PE-transposed per 128-tile.
 - exp(scores) runs on ACT straight out of PSUM with the 1/sqrt(HD) scale
   folded in; no max-subtraction (|scores| < 8 for unit-RMS inputs and
   0.02-scaled weights; fp32 exp is safe). The sigmoid gate is computed as
   1/(1+exp(-g)) so ACT never swaps activation tables in the hot loop.
"""
import math
from contextlib import ExitStack

import numpy as np

B, S_FULL, HID = 2, 2048, 2048
NH, NKV, HD = 16, 4, 128
G = NH // NKV  # 4 q heads per kv head = heads per rank
EPS = 1e-6
N_CORES = 8
P = 128
KH = HID // P  # 16 contraction tiles
HQ = HID // 4  # per-rank output column quarter (512)


def build(S=S_FULL):
    import concourse.bass as bass  # noqa: F401
    import concourse.tile as tile
    from concourse import bacc, mybir

    F32R = mybir.dt.float32r
    F32 = mybir.dt.float32
    BF16 = mybir.dt.bfloat16
    AF = mybir.ActivationFunctionType

    SQCH = S // 4  # attention sq chunk
    NW = min(512, S)  # projection free-dim chunk
    NCH = S // NW
    NSK = S // P  # score key tiles
    HPR = G * HD  # feats per rank for q/gate (512)
    SCALE = 1.0 / math.sqrt(HD)
    RG = [[0, 1, 2, 3], [4, 5, 6, 7]]

    nc = bacc.Bacc("TRN2", target_bir_lowering=False, debug=False, num_devices=N_CORES)

    hst = nc.declare_dram_parameter("hst", [HID, S], F32R, isOutput=False)
    wqt = nc.declare_dram_parameter("wqt", [HID, 2 * HPR], F32R, isOutput=False)
    wkt = nc.declare_dram_parameter("wkt", [HID, HD], F32R, isOutput=False)
    wvt = nc.declare_dram_parameter("wvt", [HID, HD], F32R, isOutput=False)
    wot = nc.declare_dram_parameter("wot", [NH * HD, HQ], F32R, isOutput=False)
    onesp = nc.declare_dram_parameter("onesp", [P, 1], F32R, isOutput=False)
    identp = nc.declare_dram_parameter("identp", [P, P], F32R, isOutput=False)
    out = nc.declare_dram_parameter("out", [HQ, S], F32, isOutput=True)

    with tile.TileContext(nc) as tc, ExitStack() as ctx:
        dram = ctx.enter_context(tc.tile_pool(name="dram", bufs=1, space="DRAM"))
        qt_dram = dram.tile([G * P, S], F32R)
        gate_dram = dram.tile([G * P, S], F32R)
        ag_in = [
            dram.tile([P, S], BF16, name=f"ag_in{h}", uniquify=False)
            for h in range(G)
        ]
        ag_out = [
            dram.tile([4 * P, S], BF16, name=f"ag_out{h}", uniquify=False)
            for h in range(G)
        ]

        # tiny warmup collective: absorbs NRT collective-channel init +
        # cross-core launch skew concurrently with the compute phases, so
        # the first real AllGather runs at steady-state cost
        # same byte-size as the real per-head AllGathers so the mesh channel
        # setup for that message size is paid here, overlapped with compute
        warm_in = dram.tile([P, S // 2], F32R)
        warm_out = dram.tile([4 * P, S // 2], F32R)
        nc.gpsimd.dma_start(out=warm_in[:], in_=hst[0:P, 0:S // 2])
        nc.gpsimd.collective_compute(
            "AllGather",
            mybir.AluOpType.bypass,
            replica_groups=RG,
            ins=[warm_in[:].opt()],
            outs=[warm_out[:].opt()],
        )
        rs_dram = dram.tile([1, S // 4], F32)

        consts = ctx.enter_context(tc.tile_pool(name="consts", bufs=1))
        ones_sb = consts.tile([P, 1], F32R)
        nc.sync.dma_start(out=ones_sb[:], in_=onesp[:])
        ident_sb = consts.tile([P, P], F32R)
        nc.sync.dma_start(out=ident_sb[:], in_=identp[:])
        rstd_bc = consts.tile([P, S], F32)
        eps_t = consts.tile([1, 1], F32)
        nc.vector.memset(eps_t[:], EPS)

        with ExitStack() as ph123:
            kv_pool = ph123.enter_context(tc.tile_pool(name="kv", bufs=1))
            kt_sb = kv_pool.tile([P, S], F32R)
            vnat = [kv_pool.tile([P, P], F32R, name=f"vnat{i}", uniquify=False)
                    for i in range(NSK)]

            # ---- phases 1+2: norm stats + projections (hsT resident) ----
            with ExitStack() as ph:
                ht_pool = ph.enter_context(tc.tile_pool(name="ht", bufs=1))
                ht = [ht_pool.tile([P, S], F32R, name=f"ht{k}", uniquify=False)
                      for k in range(KH)]
                vt_sb = ht_pool.tile([P, S], F32R)

                # norm stats + projections. The mean-square matvecs are
                # software-pipelined one k behind their squares (half on
                # ACT, half on DVE), with the m=0 q-projection psum chains
                # interleaved so the PE never idles waiting on squares.
                with tc.tile_pool(name="sqp", bufs=4) as sqp, tc.tile_pool(
                    name="wq", bufs=2
                ) as wqp, tc.tile_pool(name="ev", bufs=3) as evp:

                    def load_wq(m):
                        wq_m = wqp.tile([P, KH, P], F32R, name="wq_m", tag="wq_m")
                        for k in range(KH):
                            nc.sync.dma_start(
                                out=wq_m[:, k, :],
                                in_=wqt[k * P:(k + 1) * P, m * P:(m + 1) * P],
                            )
                        return wq_m

                    def qg_chain_mm(ps, wq_m, k, n):
                        nc.tensor.matmul(
                            ps[:],
                            wq_m[:, k, :],
                            ht[k][:, n * NW:(n + 1) * NW],
                            start=(k == 0),
                            stop=(k == KH - 1),
                        )

                    def evac(ps, m, n):
                        ev = evp.tile([P, NW], F32R)
                        nc.vector.tensor_mul(
                            ev[:], ps[:], rstd_bc[:, n * NW:(n + 1) * NW]
                        )
                        dst = qt_dram if m < G else gate_dram
                        mm = m if m < G else m - G
                        nc.sync.dma_start(
                            out=dst[mm * P:(mm + 1) * P, n * NW:(n + 1) * NW],
                            in_=ev[:],
                        )

                    qgps0_cm = tc.tile_pool(name="qgps0", bufs=1, space="PSUM")
                    qgps0 = qgps0_cm.__enter__()
                    with tc.tile_pool(name="msp", bufs=1, space="PSUM") as msp:
                        ms_ps = [msp.tile([1, NW], F32, name=f"ms{n}",
                                          uniquify=False) for n in range(NCH)]
                        wq_0 = load_wq(0)
                        for k in range(KH):
                            dma_eng = nc.sync if k % 2 == 0 else nc.scalar
                            dma_eng.dma_start(
                                out=ht[k][:], in_=hst[k * P:(k + 1) * P, :]
                            )
                        ps_m0 = [qgps0.tile([P, NW], F32, name=f"psq{n}")
                                 for n in range(NCH)]
                        sq_prev = None
                        for k in range(KH):
                            sq_k = []
                            for n in range(NCH):
                                sqk = sqp.tile([P, NW], F32R)
                                src = ht[k][:, n * NW:(n + 1) * NW]
                                if n % 2 == 0:
                                    nc.scalar.activation(sqk[:], src, AF.Square)
                                else:
                                    nc.vector.tensor_mul(sqk[:], src, src)
                                sq_k.append(sqk)
                            for n in range(NCH):
                                qg_chain_mm(ps_m0[n], wq_0, k, n)
                            if sq_prev is not None:
                                for n in range(NCH):
                                    nc.tensor.matmul(
                                        ms_ps[n][:],
                                        ones_sb[:],
                                        sq_prev[n][:],
                                        start=(k == 1),
                                        stop=(k == KH - 1 + 1),
                                    )
                            sq_prev = sq_k
                        for n in range(NCH):
                            nc.tensor.matmul(
                                ms_ps[n][:], ones_sb[:], sq_prev[n][:],
                                start=False, stop=True,
                            )
                        srow = sqp.tile([1, S], F32, bufs=1)
                        for n in range(NCH):
                            nc.scalar.activation(
                                srow[:, n * NW:(n + 1) * NW],
                                ms_ps[n][:],
                                AF.Sqrt,
                                bias=eps_t[:],
                                scale=1.0 / HID,
                            )
                        nc.vector.reciprocal(srow[:], srow[:])
                        nc.gpsimd.partition_broadcast(rstd_bc[:], srow[:])
                        for n in range(NCH):
                            evac(ps_m0[n], 0, n)
                    qgps0_cm.__exit__(None, None, None)

                    with tc.tile_pool(name="qgps", bufs=2, space="PSUM") as qgps:
                        # remaining q/gate head tiles
                        for m in range(1, 2 * G):
                            wq_m = load_wq(m)
                            for n in range(NCH):
                                ps = qgps.tile([P, NW], F32, name="psq_r")
                                for k in range(KH):
                                    qg_chain_mm(ps, wq_m, k, n)
                                evac(ps, m, n)

                    # k and v (vT), rstd-scaled at evacuation; the weight
                    # tiles share the wq_m rotation slots
                    wk_sb = wqp.tile([P, KH, P], F32R, name="wk_sb", tag="wq_m")
                    wv_sb = wqp.tile([P, KH, P], F32R, name="wv_sb", tag="wq_m")
                    for k in range(KH):
                        nc.sync.dma_start(
                            out=wk_sb[:, k, :], in_=wkt[k * P:(k + 1) * P, :]
                        )
                        nc.sync.dma_start(
                            out=wv_sb[:, k, :], in_=wvt[k * P:(k + 1) * P, :]
                        )
                    with tc.tile_pool(name="kvps", bufs=2, space="PSUM") as kvps:
                     for dst_sb, w_sb in ((kt_sb, wk_sb), (vt_sb, wv_sb)):
                        for n in range(NCH):
                            ps = kvps.tile([P, NW], F32, name="ps_kv")
                            for k in range(KH):
                                nc.tensor.matmul(
                                    ps[:],
                                    w_sb[:, k, :],
                                    ht[k][:, n * NW:(n + 1) * NW],
                                    start=(k == 0),
                                    stop=(k == KH - 1),
                                )
                            nc.vector.tensor_mul(
                                dst_sb[:, n * NW:(n + 1) * NW],
                                ps[:],
                                rstd_bc[:, n * NW:(n + 1) * NW],
                            )

                # v natural layout via PE transpose of vT tiles
                with tc.tile_pool(name="tpps", bufs=2, space="PSUM") as tpps:
                    for sk in range(NSK):
                        pst = tpps.tile([P, P], F32R)
                        nc.tensor.transpose(
                            pst[:], vt_sb[:, sk * P:(sk + 1) * P], ident_sb[:]
                        )
                        nc.vector.tensor_copy(vnat[sk][:], pst[:])

            # ---- phases 3+4 pools (allocated in the freed hsT zone) ----
            with ExitStack() as ph34:
                wo_bfp = ph34.enter_context(tc.tile_pool(name="wo_bf", bufs=1))
                wo_bf = [wo_bfp.tile([P, HQ], BF16, name=f"wo{kf}", uniquify=False)
                         for kf in range(KH)]
                of_pool = ph34.enter_context(tc.tile_pool(name="of", bufs=1))
                of = [of_pool.tile([P, S], BF16, name=f"of{i}", uniquify=False)
                      for i in range(KH)]
                with tc.tile_pool(name="wof", bufs=2) as wofp:
                    for kf in range(KH):
                        wof = wofp.tile([P, HQ], F32R)
                        nc.sync.dma_start(
                            out=wof[:], in_=wot[kf * P:(kf + 1) * P, :]
                        )
                        nc.vector.tensor_copy(wo_bf[kf][:], wof[:])

                # ---- phase 3: attention ----
                with tc.tile_pool(name="qload", bufs=3) as qlp, tc.tile_pool(
                    name="pt", bufs=3
                ) as ptp, tc.tile_pool(name="og", bufs=2) as ogp, tc.tile_pool(
                    name="sps", bufs=2, space="PSUM"
                ) as sps, tc.tile_pool(
                    name="ops", bufs=3, space="PSUM"
                ) as ops, tc.tile_pool(name="sums", bufs=1, space="PSUM") as sums:
                    for h in range(G):
                        for sqc in range(4):
                            ssl = slice(sqc * SQCH, (sqc + 1) * SQCH)
                            qtile = qlp.tile([P, SQCH], F32R)
                            nc.sync.dma_start(
                                out=qtile[:], in_=qt_dram[h * P:(h + 1) * P, ssl]
                            )
                            gtile = qlp.tile([P, SQCH], F32R)
                            nc.sync.dma_start(
                                out=gtile[:], in_=gate_dram[h * P:(h + 1) * P, ssl]
                            )
                            eng = ogp.tile([P, SQCH], F32)
                            nc.scalar.activation(eng[:], gtile[:], AF.Exp, scale=-1.0)
                            e1 = ogp.tile([P, SQCH], F32)
                            nc.vector.tensor_scalar_add(e1[:], eng[:], 1.0)
                            sig = ogp.tile([P, SQCH], F32)
                            nc.vector.reciprocal(sig[:], e1[:])
                            ps_o = ops.tile([P, SQCH], F32)
                            ps_sum = sums.tile([1, SQCH], F32)

                            # pairs of sk tiles share one 2-bank psum + one
                            # exp; p@v of pair skp-1 is emitted after the
                            # scores of pair skp so the PE never sits behind
                            # the ACT exp on the critical path
                            def emit_pv(skp, pt):
                                for j in range(2):
                                    sk = 2 * skp + j
                                    nc.tensor.matmul(
                                        ps_sum[:],
                                        ones_sb[:],
                                        pt[:, j, :],
                                        start=(sk == 0),
                                        stop=(sk == NSK - 1),
                                    )
                                    nc.tensor.matmul(
                                        ps_o[:],
                                        vnat[sk][:],
                                        pt[:, j, :],
                                        start=(sk == 0),
                                        stop=(sk == NSK - 1),
                                    )

                            pend = None
                            for skp in range(NSK // 2):
                                ps_s = sps.tile([P, 2, SQCH], F32)
                                for j in range(2):
                                    sk = 2 * skp + j
                                    nc.tensor.matmul(
                                        ps_s[:, j, :],
                                        kt_sb[:, sk * P:(sk + 1) * P],
                                        qtile[:],
                                        start=True,
                                        stop=True,
                                    )
                                pt = ptp.tile([P, 2, SQCH], F32R)
                                nc.scalar.activation(
                                    pt[:], ps_s[:], AF.Exp, scale=SCALE
                                )
                                if pend is not None:
                                    emit_pv(*pend)
                                pend = (skp, pt)
                            emit_pv(*pend)

                            # normalize + sigmoid gate (exp-form, no ACT
                            # table swap: sigmoid(g) = 1/(1+exp(-g)))
                            rs = ogp.tile([1, SQCH], F32)
                            nc.vector.reciprocal(rs[:], ps_sum[:])
                            # broadcast 1/sum along partitions via a DRAM
                            # round-trip on the sync DGE (gpsimd must stay
                            # free to trigger collectives)
                            nc.sync.dma_start(out=rs_dram[:, :SQCH], in_=rs[:])
                            rb = ogp.tile([P, SQCH], F32)
                            nc.sync.dma_start(
                                out=rb[:],
                                in_=rs_dram[:, :SQCH].to_broadcast((P, SQCH)),
                            )
                            # multiply by the gate FIRST: releases the ps_o
                            # psum slot without waiting on the rb DMA trip
                            t1 = ogp.tile([P, SQCH], F32)
                            nc.vector.tensor_mul(t1[:], ps_o[:], sig[:])
                            og = ogp.tile([P, SQCH], BF16)
                            nc.vector.tensor_mul(og[:], t1[:], rb[:])
                            nc.sync.dma_start(out=ag_in[h][:, ssl], in_=og[:])
                        nc.gpsimd.collective_compute(
                            "AllGather",
                            mybir.AluOpType.bypass,
                            replica_groups=RG,
                            ins=[ag_in[h][:].opt()],
                            outs=[ag_out[h][:].opt()],
                        )
                        # off the sync queue: these wait on the AllGather and
                        # must not head-of-line-block the q/gate reloads
                        for r in range(4):
                            nc.gpsimd.dma_start(
                                out=of[h * 4 + r][:],
                                in_=ag_out[h][r * P:(r + 1) * P, :],
                            )

                # ---- phase 4: O projection (bf16), my HID column quarter ----
                # kf-outer accumulation so heads 0..2 contract while the
                # last AllGather is still in flight
                with tc.tile_pool(name="outps", bufs=2, space="PSUM") as outps, \
                        tc.tile_pool(name="oev", bufs=3) as oevp:
                    NM = HQ // P
                    for n in range(NCH):
                        pss = [outps.tile([P, NW], F32, name=f"ops{m}")
                               for m in range(NM)]
                        for kf in range(KH):
                            for m in range(NM):
                                nc.tensor.matmul(
                                    pss[m][:],
                                    wo_bf[kf][:, m * P:(m + 1) * P],
                                    of[kf][:, n * NW:(n + 1) * NW],
                                    start=(kf == 0),
                                    stop=(kf == KH - 1),
                                )
                        for m in range(NM):
                            oev = oevp.tile([P, NW], F32)
                            nc.vector.tensor_copy(oev[:], pss[m][:])
                            nc.sync.dma_start(
                                out=out[m * P:(m + 1) * P, n * NW:(n + 1) * NW],
                                in_=oev[:],
                            )

    nc.compile()
    return nc


def make_in_maps(hidden_states, Wq, Wk, Wv, Wo, norm_w, S=S_FULL):
    """Host-side sharding/layout prep. Core c -> (batch c//4, rank c%4)."""
    w1p = (1.0 + norm_w).astype(np.float32)
    WqT = np.ascontiguousarray((Wq * w1p[None, :]).T)  # [HID, 2*NH*HD]
    WkT = np.ascontiguousarray((Wk * w1p[None, :]).T)  # [HID, NKV*HD]
    WvT = np.ascontiguousarray((Wv * w1p[None, :]).T)
    WoT = np.ascontiguousarray(Wo.T)  # [NH*HD, HID]
    # permute feat blocks to match AG stacking: pos h*4+r holds head 4r+h
    perm = [4 * (p % 4) + p // 4 for p in range(NH)]
    WoTp = np.ascontiguousarray(
        WoT.reshape(NH, HD, HID)[perm].reshape(NH * HD, HID)
    )
    ones = np.ones((P, 1), np.float32)
    ident = np.eye(P, dtype=np.float32)

    in_maps = []
    for c in range(N_CORES):
        b, r = c // 4, c % 4
        qcols = np.r_[r * 512:(r + 1) * 512, NH * HD + r * 512:NH * HD + (r + 1) * 512]
        in_maps.append(
            {
                "hst": np.ascontiguousarray(hidden_states[b, :S].T),
                "wqt": np.ascontiguousarray(WqT[:, qcols]),
                "wkt": np.ascontiguousarray(WkT[:, r * HD:(r + 1) * HD]),
                "wvt": np.ascontiguousarray(WvT[:, r * HD:(r + 1) * HD]),
                "wot": np.ascontiguousarray(WoTp[:, r * HQ:(r + 1) * HQ]),
                "onesp": ones,
                "identp": ident,
            }
        )
    return in_maps


def gather_out(results, S=S_FULL):
    out = np.empty((B, S, HID), np.float32)
    for c in range(N_CORES):
        b, r = c // 4, c % 4
        out[b, :, r * HQ:(r + 1) * HQ] = results[c]["out"].T
    return out


_NC_CACHE = {}


def kernel(**inputs) -> np.ndarray:
    from concourse.bass_utils import run_bass_kernel_spmd

    hidden_states = np.asarray(inputs["hidden_states"], dtype=np.float32)
    Wq = np.asarray(inputs["Wq"], dtype=np.float32)
    Wk = np.asarray(inputs["Wk"], dtype=np.float32)
    Wv = np.asarray(inputs["Wv"], dtype=np.float32)
    Wo = np.asarray(inputs["Wo"], dtype=np.float32)
    norm_w = np.asarray(inputs["norm_w"], dtype=np.float32)

    if "nc" not in _NC_CACHE:
        _NC_CACHE["nc"] = build()
    nc = _NC_CACHE["nc"]

    in_maps = make_in_maps(hidden_states, Wq, Wk, Wv, Wo, norm_w)
    res = run_bass_kernel_spmd(nc, in_maps, list(range(N_CORES)))
    return gather_out(res.results)
